# revision 29
# baseline (speedup 1.0000x reference)
"""MiniMax MoE gate (sigmoid + bias, top-8, normalized weights) on 8 TRN2 cores.

Full inputs in, full outputs out. Data-parallel over tokens: each core gets
1024 of the 8192 tokens; gate weight and bias are replicated.

Strategy (fp16+fp8 compensated, 12.6MB/core of x traffic vs 16.8MB for the
bf16 hi/lo 3-term scheme; validated metric 8.4e-3 vs 2.5e-3 full-fp32,
gate 2e-2):
  - x ships as fp16 hi (8.4MB) + e4m3 residual scaled 2^12 (4.2MB):
    x = hi + 2^-12*deq(r8) to ~2^-16 relative. W rides as a packed fp16
    [Whi | Wlo] moving operand (W-side ~fp32 exact) plus an e4m3 copy
    (x 2^6) for the residual term.
  - Per 128-token tile: psum1[128,128] = x16^T@[Whi|Wlo] over 32 chunks
    (LDW fp16 FWL + N=128 moving), psum2[128,64] = r8^T@W8 (fp8). Per
    group, all term1 matmuls run before all term2 so the r8 stream (on
    the other queue) gets extra arrival slack. logits = psum1[:,:64] +
    psum1[:,64:] + 2^-18*psum2, with the fp8 psum pre-scaled to SBUF on
    ACT (DVE ops may read at most one PSUM operand).
  - DMA: the HWDGE emits ~1 descriptor per partition per dma_start at
    ~25ns each, so per-partition runs are kept >=8KB. xh alternates the
    sync/scalar HWDGE queues per group (16-chunk halves for chase
    granularity), r8 rides the opposite queue, wpk chunk-halves split
    across both so the first matmuls unblock early; outputs + bias ride
    gpsimd. Measured: both queues together sustain the ~350GB/s/core
    chip-HBM-limited rate; x DMA is the critical path.
  - ACT LUTs are pre-warmed after the DMA triggers are queued (a lazy
    sigmoid table load used to stall the PE mid-kernel for ~4.6us).
  - top-8 via DVE MAX8/FIND_INDEX8 (descending, ties by ascending index
    - exactly jax.lax.top_k). Selected raw scores via a bf16 one-hot
    against an expert-id iota (indices are bf16-exact, 2x DVE rate, no
    fp32 tie hazard): ssel = sum_e (idx==e)*sigmoid_bf16; weights =
    ssel * recip(sum). Per-group psum/output pools are deep enough that
    buffer recycling never blocks the PE on the DVE epilogue.
"""

import os

import numpy as np
import ml_dtypes

import concourse.bacc as bacc
import concourse.mybir as mybir
from concourse.bass_utils import run_bass_kernel_spmd
from concourse.tile import TileContext

T, D, E, K = 8192, 4096, 64, 8
NCORES = 8
P = 128
F32 = mybir.dt.float32
FP16 = mybir.dt.float16
FP8 = mybir.dt.float8e4
BF16 = mybir.dt.bfloat16
FP16_NP = np.float16
FP8_NP = ml_dtypes.float8_e4m3fn
DC = D // P   # 32 contraction chunks

R_SCALE = 2.0 ** 12   # residual premultiplier before e4m3 quantization
W8_SCALE = 2.0 ** 6   # W premultiplier for the fp8 copy
FOLD = 1.0 / (R_SCALE * W8_SCALE)

# tiles (x128 tokens) per token-group, per core
GROUP_PLAN = tuple(
    int(v) for v in os.environ.get("KPLAN", "2,2,2,2").split(",")
)
GATHER = os.environ.get("KGATHER", "1") == "1"


def _plans(ts):
    nt = ts // P
    plan = list(GROUP_PLAN)
    if sum(plan) != nt:  # fallback for small test shards
        plan = [1] * nt
    return plan


def build_nc(ts):
    """Per-core program for a shard of `ts` tokens."""
    plan = _plans(ts)
    nh = len(plan)
    ths = [p * P for p in plan]
    total_cols = DC * sum(ths)

    nc = bacc.Bacc("TRN2", target_bir_lowering=False)
    # host-tiled layout (see prepare_in_maps): group blocks side by side;
    # within a group, row p holds all of partition p's data contiguously.
    xhd = nc.dram_tensor("xt_hi", [P, total_cols], FP16, kind="ExternalInput")
    xrd = nc.dram_tensor("xt_r8", [P, total_cols], FP8, kind="ExternalInput")
    wpd = nc.dram_tensor("wt_pk", [P, DC * 2 * E], FP16, kind="ExternalInput")
    w8d = nc.dram_tensor("wt_8", [P, DC * E], FP8, kind="ExternalInput")
    b = nc.dram_tensor("bias", [1, E], F32, kind="ExternalInput")
    oi = nc.dram_tensor("out_idx", [ts, K], mybir.dt.int32, kind="ExternalOutput")
    ow = nc.dram_tensor("out_w", [ts, K], F32, kind="ExternalOutput")

    with TileContext(nc) as tc:
        with (
            tc.tile_pool(name="const", bufs=1) as cpool,
            tc.tile_pool(name="xin", bufs=1) as xpool,
            tc.tile_pool(name="epi", bufs=6) as epool,
            tc.tile_pool(name="outb", bufs=5) as opool,
            tc.tile_pool(name="plg1", bufs=4, space="PSUM") as plg1,
            tc.tile_pool(name="plg2", bufs=4, space="PSUM") as plg2,
        ):
            # W first: wpk chunk-halves split across both queues so the
            # first matmuls (which only need low chunks) unblock early;
            # bias via gpsimd
            hc = DC // 2
            wpk = cpool.tile([P, DC, 2 * E], FP16)
            nc.scalar.dma_start(
                out=wpk[:, 0:hc, :], in_=wpd[:, 0:hc * 2 * E])
            w8 = cpool.tile([P, DC, E], FP8)
            nc.scalar.dma_start(out=w8, in_=w8d[:, :])
            bias_row = cpool.tile([1, E], F32)
            nc.gpsimd.dma_start(out=bias_row, in_=b[:, :])
            bias_bc = cpool.tile([P, E], F32)
            nc.gpsimd.partition_broadcast(bias_bc, bias_row)
            # expert-id iota row, replicated: iota_bc[p, e] = e (bf16-exact)
            iota_bc = cpool.tile([P, E], BF16)
            nc.gpsimd.iota(iota_bc, pattern=[[1, E]], base=0, channel_multiplier=0,
                           allow_small_or_imprecise_dtypes=True)

            # x loads: xh alternates sync/scalar per group, r8 rides the
            # vector queue; every dma_start keeps >=8KB/partition runs so
            # the ~25ns/descriptor HWDGE generator stays off the critical
            # path. All triggers are queued up front.
            xhts, xrts = [], []
            off = 0
            for h in range(nh):
                th = ths[h]
                xht = xpool.tile([P, DC, th], FP16, tag=f"xh{h}", name=f"xh{h}")
                xrt = xpool.tile([P, DC, th], FP8, tag=f"xr{h}", name=f"xr{h}")
                q16 = nc.sync if h % 2 == 0 else nc.scalar
                q8 = nc.scalar if h % 2 == 0 else nc.sync
                if h == nh - 1 and h % 2 == 0:
                    q8 = nc.sync  # balance: sync queue runs ~10% faster
                subs = (DC,) if th == P else (DC // 2, DC // 2)
                c0 = 0
                for cs in subs:
                    sl = slice(c0 * th, (c0 + cs) * th)
                    dsl = slice(off + c0 * th, off + (c0 + cs) * th)
                    q16.dma_start(
                        out=xht[:].rearrange("p c t -> p (c t)")[:, sl],
                        in_=xhd[:, dsl],
                    )
                    c0 += cs
                if h == 0:
                    nc.sync.dma_start(
                        out=wpk[:, hc:DC, :], in_=wpd[:, hc * 2 * E:])
                q8.dma_start(
                    out=xrt[:].rearrange("p c t -> p (c t)"),
                    in_=xrd[:, off:off + DC * th],
                )
                off += DC * th
                xhts.append(xht)
                xrts.append(xrt)

            # warm the ACT LUTs (sigmoid + copy) after the DMA triggers are
            # queued but long before the first real sigmoid (a lazy table
            # load used to stall the PE mid-kernel for ~4.6us)
            warm = cpool.tile([1, 8], F32)
            nc.vector.memset(warm, 0.0)
            warm2 = cpool.tile([1, 8], F32)
            nc.scalar.activation(
                out=warm2, in_=warm,
                func=mybir.ActivationFunctionType.Sigmoid,
            )
            nc.scalar.copy(out=warm, in_=warm2)

            tok0 = 0
            for h in range(nh):
                th, ntg = ths[h], plan[h]
                xht, xrt = xhts[h], xrts[h]

                oidx = opool.tile(
                    [P, ntg, K], mybir.dt.uint32, tag=f"oidx{ntg}", name=f"oidx{h}"
                )
                owgt = opool.tile(
                    [P, ntg, K], F32, tag=f"owgt{ntg}", name=f"owgt{h}"
                )
                # all term1 matmuls for the group first, then all term2:
                # the PE drains in program order, so this gives the r8
                # stream (on the other queue) an extra term1's worth of
                # slack before the PE needs it
                lg1s, lg2s, tmps = [], [], []

                def emit_term1():
                    for i in range(ntg):
                        tsl = slice(i * P, (i + 1) * P)
                        lg1 = plg1.tile(
                            [P, 2 * E], F32, tag="lg1", name=f"lg1_h{h}_{i}")
                        for c in range(DC):
                            nc.tensor.matmul(
                                lg1, xht[:, c, tsl], wpk[:, c, :],
                                start=(c == 0), stop=(c == DC - 1),
                            )
                        lg1s.append(lg1)

                def emit_term2():
                    for i in range(ntg):
                        tsl = slice(i * P, (i + 1) * P)
                        lg2 = plg2.tile(
                            [P, E], F32, tag="lg2", name=f"lg2_h{h}_{i}")
                        for c in range(DC):
                            nc.tensor.matmul(
                                lg2, xrt[:, c, tsl], w8[:, c, :],
                                start=(c == 0), stop=(c == DC - 1),
                            )
                        lg2s.append(lg2)
                        tmp = epool.tile([P, E], F32, tag="tmp")
                        nc.scalar.mul(tmp, lg2, FOLD)
                        tmps.append(tmp)

                emit_term1()
                emit_term2()
                for i in range(ntg):
                    lg1 = lg1s[i]
                    tmp = tmps[i]

                    # fold: logits = lg1[:, :E] + lg1[:, E:] + tmp
                    # (tmp = FOLD*lg2, staged to SBUF on ACT; one PSUM
                    # operand per DVE op)
                    t2 = epool.tile([P, E], F32, tag="t2")
                    nc.vector.tensor_tensor(
                        out=t2, in0=tmp, in1=lg1[:, 0:E],
                        op=mybir.AluOpType.add,
                    )
                    lgs = epool.tile([P, E], F32, tag="lgs")
                    nc.vector.tensor_tensor(
                        out=lgs, in0=t2, in1=lg1[:, E:2 * E],
                        op=mybir.AluOpType.add,
                    )
                    sc = epool.tile([P, E], F32, tag="sc")
                    nc.scalar.activation(
                        out=sc, in_=lgs,
                        func=mybir.ActivationFunctionType.Sigmoid,
                    )
                    scb = epool.tile([P, E], BF16, tag="scb")
                    nc.scalar.copy(out=scb, in_=sc)
                    bi = epool.tile([P, E], F32, tag="bi")
                    nc.vector.tensor_tensor(
                        out=bi, in0=sc, in1=bias_bc, op=mybir.AluOpType.add
                    )
                    msel = epool.tile([P, K], F32, tag="msel")
                    nc.vector.max(out=msel, in_=bi)
                    nc.vector.max_index(
                        out=oidx[:, i, :], in_max=msel, in_values=bi
                    )
                    # one-hot by expert INDEX (exact in bf16, 2x DVE rate,
                    # and immune to fp32 biased-score ties)
                    idxb = epool.tile([P, K], BF16, tag="idxb")
                    nc.vector.tensor_copy(out=idxb, in_=oidx[:, i, :])
                    idx_b = idxb[:].rearrange(
                        "p (k o) -> p k o", o=1
                    ).to_broadcast([P, K, E])
                    iota_b = iota_bc[:].rearrange(
                        "p (o e) -> p o e", o=1
                    ).to_broadcast([P, K, E])
                    scb_b = scb[:].rearrange(
                        "p (o e) -> p o e", o=1
                    ).to_broadcast([P, K, E])
                    oh8 = epool.tile([P, K, E], BF16, tag="oh8")
                    nc.vector.tensor_tensor(
                        out=oh8, in0=idx_b, in1=iota_b,
                        op=mybir.AluOpType.is_equal,
                    )
                    ohs = epool.tile([P, K, E], BF16, tag="ohs")
                    nc.vector.tensor_tensor(
                        out=ohs, in0=oh8, in1=scb_b, op=mybir.AluOpType.mult
                    )
                    ssel = epool.tile([P, K], F32, tag="ssel")
                    nc.vector.tensor_reduce(
                        out=ssel, in_=ohs,
                        axis=mybir.AxisListType.X, op=mybir.AluOpType.add,
                    )
                    ssum = epool.tile([P, 1], F32, tag="ssum")
                    nc.vector.tensor_reduce(
                        out=ssum, in_=ssel,
                        axis=mybir.AxisListType.X, op=mybir.AluOpType.add,
                    )
                    rsum = epool.tile([P, 1], F32, tag="rsum")
                    nc.vector.reciprocal(out=rsum, in_=ssum)
                    nc.vector.tensor_scalar_mul(owgt[:, i, :], ssel, rsum[:])

                # token at output partition q of tile i is tok0 + q*ntg + i
                nc.gpsimd.dma_start(
                    out=oi[tok0:tok0 + th, :].rearrange(
                        "(q i) k -> q i k", i=ntg
                    ),
                    in_=oidx[:].bitcast(mybir.dt.int32),
                )
                nc.gpsimd.dma_start(
                    out=ow[tok0:tok0 + th, :].rearrange(
                        "(q i) k -> q i k", i=ntg
                    ),
                    in_=owgt,
                )
                tok0 += th

    nc.compile()
    return nc


_NC_CACHE = {}


def _get_nc(ts):
    if ts not in _NC_CACHE:
        _NC_CACHE[ts] = build_nc(ts)
    return _NC_CACHE[ts]


def _tile_xt(xs, ts):
    """[ts, D] fp32 -> [P, DC*ts] fp32 in the device layout.

    Groups laid side by side; within group h (tiles ntg, tokens th=128*ntg),
    flat column off_h + c*th + i*P + q holds x[tok0 + q*ntg + i, c*P + p]
    at partition row p.
    """
    plan = _plans(ts)
    blocks = []
    tok0 = 0
    for ntg in plan:
        th = ntg * P
        a = xs[tok0:tok0 + th].reshape(P, ntg, DC, P)  # [q, i, c, p]
        a = a.transpose(3, 2, 1, 0)                    # [p, c, i, q]
        blocks.append(np.ascontiguousarray(a).reshape(P, DC * th))
        tok0 += th
    return np.concatenate(blocks, axis=1)


def prepare_in_maps(x, gate_weight, bias):
    x = np.asarray(x, dtype=np.float32)
    gw = np.asarray(gate_weight, dtype=np.float32)
    bb = np.ascontiguousarray(np.asarray(bias, dtype=np.float32)).reshape(1, E)

    ts = T // NCORES

    # W^T in device layout [P, DC, E]: [p, c, e] = W[e, c*P + p]
    wt = np.ascontiguousarray(gw.T.reshape(DC, P, E).transpose(1, 0, 2))
    wh = wt.astype(FP16_NP)
    wl = (wt - wh.astype(np.float32)).astype(FP16_NP)
    wpk = np.concatenate([wh, wl], axis=2).reshape(P, DC * 2 * E)
    w8 = (wt * W8_SCALE).astype(FP8_NP).reshape(P, DC * E)

    in_maps = []
    for cid in range(NCORES):
        xt = _tile_xt(x[cid * ts:(cid + 1) * ts], ts)
        xh = xt.astype(FP16_NP)
        xr = ((xt - xh.astype(np.float32)) * R_SCALE).astype(FP8_NP)
        in_maps.append({
            "xt_hi": xh,
            "xt_r8": xr,
            "wt_pk": wpk,
            "wt_8": w8,
            "bias": bb,
        })
    return in_maps


def kernel(x, gate_weight, bias):
    ts = T // NCORES
    nc = _get_nc(ts)
    in_maps = prepare_in_maps(x, gate_weight, bias)
    res = run_bass_kernel_spmd(nc, in_maps, core_ids=list(range(NCORES)))
    idx = np.concatenate([r["out_idx"] for r in res.results], axis=0)
    wts = np.concatenate([r["out_w"] for r in res.results], axis=0)
    return idx, wts


# revision 30
# speedup vs baseline: 1.0049x; 1.0049x over previous
"""MiniMax MoE gate (sigmoid + bias, top-8, normalized weights) on 8 TRN2 cores.

Full inputs in, full outputs out. Data-parallel over tokens: each core gets
1024 of the 8192 tokens; gate weight and bias are replicated.

Strategy (fp16+fp8 compensated, 12.6MB/core of x traffic vs 16.8MB for the
bf16 hi/lo 3-term scheme; validated metric 8.4e-3 vs 2.5e-3 full-fp32,
gate 2e-2):
  - x ships as fp16 hi (8.4MB) + e4m3 residual scaled 2^12 (4.2MB):
    x = hi + 2^-12*deq(r8) to ~2^-16 relative. W rides as a packed fp16
    [Whi | Wlo] moving operand (W-side ~fp32 exact) plus an e4m3 copy
    (x 2^6) for the residual term.
  - Per 128-token tile: psum1[128,128] = x16^T@[Whi|Wlo] over 32 chunks
    (LDW fp16 FWL + N=128 moving), psum2[128,64] = r8^T@W8 (fp8). Per
    group, all term1 matmuls run before all term2 so the r8 stream (on
    the other queue) gets extra arrival slack. logits = psum1[:,:64] +
    psum1[:,64:] + 2^-18*psum2, with the fp8 psum pre-scaled to SBUF on
    ACT (DVE ops may read at most one PSUM operand).
  - DMA: the HWDGE emits ~1 descriptor per partition per dma_start at
    ~25ns each, so per-partition runs are kept >=8KB. xh alternates the
    sync/scalar HWDGE queues per group (16-chunk halves for chase
    granularity), r8 rides the opposite queue, wpk chunk-halves split
    across both so the first matmuls unblock early; outputs + bias ride
    gpsimd. Measured: both queues together sustain the ~350GB/s/core
    chip-HBM-limited rate; x DMA is the critical path.
  - ACT LUTs are pre-warmed after the DMA triggers are queued (a lazy
    sigmoid table load used to stall the PE mid-kernel for ~4.6us).
  - top-8 via DVE MAX8/FIND_INDEX8 (descending, ties by ascending index
    - exactly jax.lax.top_k). Selected raw scores via a bf16 one-hot
    against an expert-id iota (indices are bf16-exact, 2x DVE rate, no
    fp32 tie hazard): ssel = sum_e (idx==e)*sigmoid_bf16; weights =
    ssel * recip(sum). Per-group psum/output pools are deep enough that
    buffer recycling never blocks the PE on the DVE epilogue.
"""

import os

import numpy as np
import ml_dtypes

import concourse.bacc as bacc
import concourse.mybir as mybir
from concourse.bass_utils import run_bass_kernel_spmd
from concourse.tile import TileContext

T, D, E, K = 8192, 4096, 64, 8
NCORES = 8
P = 128
F32 = mybir.dt.float32
FP16 = mybir.dt.float16
FP8 = mybir.dt.float8e4
BF16 = mybir.dt.bfloat16
FP16_NP = np.float16
FP8_NP = ml_dtypes.float8_e4m3fn
DC = D // P   # 32 contraction chunks

R_SCALE = 2.0 ** 12   # residual premultiplier before e4m3 quantization
W8_SCALE = 2.0 ** 6   # W premultiplier for the fp8 copy
FOLD = 1.0 / (R_SCALE * W8_SCALE)

# tiles (x128 tokens) per token-group, per core
GROUP_PLAN = tuple(
    int(v) for v in os.environ.get("KPLAN", "2,2,2,2").split(",")
)
GATHER = os.environ.get("KGATHER", "1") == "1"


def _plans(ts):
    nt = ts // P
    plan = list(GROUP_PLAN)
    if sum(plan) != nt:  # fallback for small test shards
        plan = [1] * nt
    return plan


def build_nc(ts):
    """Per-core program for a shard of `ts` tokens."""
    plan = _plans(ts)
    nh = len(plan)
    ths = [p * P for p in plan]
    total_cols = DC * sum(ths)

    nc = bacc.Bacc("TRN2", target_bir_lowering=False)
    # host-tiled layout (see prepare_in_maps): group blocks side by side;
    # within a group, row p holds all of partition p's data contiguously.
    xhd = nc.dram_tensor("xt_hi", [P, total_cols], FP16, kind="ExternalInput")
    xrd = nc.dram_tensor("xt_r8", [P, total_cols], FP8, kind="ExternalInput")
    wpd = nc.dram_tensor("wt_pk", [P, DC * 2 * E], FP16, kind="ExternalInput")
    w8d = nc.dram_tensor("wt_8", [P, DC * E], FP8, kind="ExternalInput")
    b = nc.dram_tensor("bias", [1, E], F32, kind="ExternalInput")
    oi = nc.dram_tensor("out_idx", [ts, K], mybir.dt.int32, kind="ExternalOutput")
    ow = nc.dram_tensor("out_w", [ts, K], F32, kind="ExternalOutput")

    with TileContext(nc) as tc:
        with (
            tc.tile_pool(name="const", bufs=1) as cpool,
            tc.tile_pool(name="xin", bufs=1) as xpool,
            tc.tile_pool(name="epi", bufs=6) as epool,
            tc.tile_pool(name="outb", bufs=5) as opool,
            tc.tile_pool(name="plg1", bufs=4, space="PSUM") as plg1,
            tc.tile_pool(name="plg2", bufs=4, space="PSUM") as plg2,
        ):
            # W first: wpk chunk-halves split across both queues so the
            # first matmuls (which only need low chunks) unblock early;
            # bias via gpsimd
            hc = DC // 2
            wpk = cpool.tile([P, DC, 2 * E], FP16)
            nc.scalar.dma_start(
                out=wpk[:, 0:hc, :], in_=wpd[:, 0:hc * 2 * E])
            w8 = cpool.tile([P, DC, E], FP8)
            nc.scalar.dma_start(out=w8, in_=w8d[:, :])
            nc.sync.dma_start(
                out=wpk[:, hc:DC, :], in_=wpd[:, hc * 2 * E:])
            bias_row = cpool.tile([1, E], F32)
            nc.gpsimd.dma_start(out=bias_row, in_=b[:, :])
            bias_bc = cpool.tile([P, E], F32)
            nc.gpsimd.partition_broadcast(bias_bc, bias_row)
            # expert-id iota row, replicated: iota_bc[p, e] = e (bf16-exact)
            iota_bc = cpool.tile([P, E], BF16)
            nc.gpsimd.iota(iota_bc, pattern=[[1, E]], base=0, channel_multiplier=0,
                           allow_small_or_imprecise_dtypes=True)

            # x loads: xh alternates sync/scalar per group, r8 rides the
            # vector queue; every dma_start keeps >=8KB/partition runs so
            # the ~25ns/descriptor HWDGE generator stays off the critical
            # path. All triggers are queued up front.
            xhts, xrts = [], []
            off = 0
            for h in range(nh):
                th = ths[h]
                xht = xpool.tile([P, DC, th], FP16, tag=f"xh{h}", name=f"xh{h}")
                xrt = xpool.tile([P, DC, th], FP8, tag=f"xr{h}", name=f"xr{h}")
                q16 = nc.sync if h % 2 == 0 else nc.scalar
                q8 = nc.scalar if h % 2 == 0 else nc.sync
                if h == nh - 1 and h % 2 == 0:
                    q8 = nc.sync  # balance: sync queue runs ~10% faster
                subs = (DC,) if th == P else (DC // 2, DC // 2)
                c0 = 0
                for cs in subs:
                    sl = slice(c0 * th, (c0 + cs) * th)
                    dsl = slice(off + c0 * th, off + (c0 + cs) * th)
                    q16.dma_start(
                        out=xht[:].rearrange("p c t -> p (c t)")[:, sl],
                        in_=xhd[:, dsl],
                    )
                    c0 += cs
                q8.dma_start(
                    out=xrt[:].rearrange("p c t -> p (c t)"),
                    in_=xrd[:, off:off + DC * th],
                )
                off += DC * th
                xhts.append(xht)
                xrts.append(xrt)

            # warm the ACT LUTs (sigmoid + copy) after the DMA triggers are
            # queued but long before the first real sigmoid (a lazy table
            # load used to stall the PE mid-kernel for ~4.6us)
            warm = cpool.tile([1, 8], F32)
            nc.vector.memset(warm, 0.0)
            warm2 = cpool.tile([1, 8], F32)
            nc.scalar.activation(
                out=warm2, in_=warm,
                func=mybir.ActivationFunctionType.Sigmoid,
            )
            nc.scalar.copy(out=warm, in_=warm2)

            tok0 = 0
            for h in range(nh):
                th, ntg = ths[h], plan[h]
                xht, xrt = xhts[h], xrts[h]

                oidx = opool.tile(
                    [P, ntg, K], mybir.dt.uint32, tag=f"oidx{ntg}", name=f"oidx{h}"
                )
                owgt = opool.tile(
                    [P, ntg, K], F32, tag=f"owgt{ntg}", name=f"owgt{h}"
                )
                # all term1 matmuls for the group first, then all term2:
                # the PE drains in program order, so this gives the r8
                # stream (on the other queue) an extra term1's worth of
                # slack before the PE needs it
                lg1s, lg2s, tmps = [], [], []

                def emit_term1():
                    for i in range(ntg):
                        tsl = slice(i * P, (i + 1) * P)
                        lg1 = plg1.tile(
                            [P, 2 * E], F32, tag="lg1", name=f"lg1_h{h}_{i}")
                        for c in range(DC):
                            nc.tensor.matmul(
                                lg1, xht[:, c, tsl], wpk[:, c, :],
                                start=(c == 0), stop=(c == DC - 1),
                            )
                        lg1s.append(lg1)

                def emit_term2():
                    for i in range(ntg):
                        tsl = slice(i * P, (i + 1) * P)
                        lg2 = plg2.tile(
                            [P, E], F32, tag="lg2", name=f"lg2_h{h}_{i}")
                        for c in range(DC):
                            nc.tensor.matmul(
                                lg2, xrt[:, c, tsl], w8[:, c, :],
                                start=(c == 0), stop=(c == DC - 1),
                            )
                        lg2s.append(lg2)
                        tmp = epool.tile([P, E], F32, tag="tmp")
                        nc.scalar.mul(tmp, lg2, FOLD)
                        tmps.append(tmp)

                emit_term1()
                emit_term2()
                for i in range(ntg):
                    lg1 = lg1s[i]
                    tmp = tmps[i]

                    # fold: logits = lg1[:, :E] + lg1[:, E:] + tmp
                    # (tmp = FOLD*lg2, staged to SBUF on ACT; one PSUM
                    # operand per DVE op)
                    t2 = epool.tile([P, E], F32, tag="t2")
                    nc.vector.tensor_tensor(
                        out=t2, in0=tmp, in1=lg1[:, 0:E],
                        op=mybir.AluOpType.add,
                    )
                    lgs = epool.tile([P, E], F32, tag="lgs")
                    nc.vector.tensor_tensor(
                        out=lgs, in0=t2, in1=lg1[:, E:2 * E],
                        op=mybir.AluOpType.add,
                    )
                    sc = epool.tile([P, E], F32, tag="sc")
                    nc.scalar.activation(
                        out=sc, in_=lgs,
                        func=mybir.ActivationFunctionType.Sigmoid,
                    )
                    scb = epool.tile([P, E], BF16, tag="scb")
                    nc.scalar.copy(out=scb, in_=sc)
                    bi = epool.tile([P, E], F32, tag="bi")
                    nc.vector.tensor_tensor(
                        out=bi, in0=sc, in1=bias_bc, op=mybir.AluOpType.add
                    )
                    msel = epool.tile([P, K], F32, tag="msel")
                    nc.vector.max(out=msel, in_=bi)
                    nc.vector.max_index(
                        out=oidx[:, i, :], in_max=msel, in_values=bi
                    )
                    # one-hot by expert INDEX (exact in bf16, 2x DVE rate,
                    # and immune to fp32 biased-score ties)
                    idxb = epool.tile([P, K], BF16, tag="idxb")
                    nc.vector.tensor_copy(out=idxb, in_=oidx[:, i, :])
                    idx_b = idxb[:].rearrange(
                        "p (k o) -> p k o", o=1
                    ).to_broadcast([P, K, E])
                    iota_b = iota_bc[:].rearrange(
                        "p (o e) -> p o e", o=1
                    ).to_broadcast([P, K, E])
                    scb_b = scb[:].rearrange(
                        "p (o e) -> p o e", o=1
                    ).to_broadcast([P, K, E])
                    oh8 = epool.tile([P, K, E], BF16, tag="oh8")
                    nc.vector.tensor_tensor(
                        out=oh8, in0=idx_b, in1=iota_b,
                        op=mybir.AluOpType.is_equal,
                    )
                    ohs = epool.tile([P, K, E], BF16, tag="ohs")
                    nc.vector.tensor_tensor(
                        out=ohs, in0=oh8, in1=scb_b, op=mybir.AluOpType.mult
                    )
                    ssel = epool.tile([P, K], F32, tag="ssel")
                    nc.vector.tensor_reduce(
                        out=ssel, in_=ohs,
                        axis=mybir.AxisListType.X, op=mybir.AluOpType.add,
                    )
                    ssum = epool.tile([P, 1], F32, tag="ssum")
                    nc.vector.tensor_reduce(
                        out=ssum, in_=ssel,
                        axis=mybir.AxisListType.X, op=mybir.AluOpType.add,
                    )
                    rsum = epool.tile([P, 1], F32, tag="rsum")
                    nc.vector.reciprocal(out=rsum, in_=ssum)
                    nc.vector.tensor_scalar_mul(owgt[:, i, :], ssel, rsum[:])

                # token at output partition q of tile i is tok0 + q*ntg + i
                nc.gpsimd.dma_start(
                    out=oi[tok0:tok0 + th, :].rearrange(
                        "(q i) k -> q i k", i=ntg
                    ),
                    in_=oidx[:].bitcast(mybir.dt.int32),
                )
                nc.gpsimd.dma_start(
                    out=ow[tok0:tok0 + th, :].rearrange(
                        "(q i) k -> q i k", i=ntg
                    ),
                    in_=owgt,
                )
                tok0 += th

    nc.compile()
    return nc


_NC_CACHE = {}


def _get_nc(ts):
    if ts not in _NC_CACHE:
        _NC_CACHE[ts] = build_nc(ts)
    return _NC_CACHE[ts]


def _tile_xt(xs, ts):
    """[ts, D] fp32 -> [P, DC*ts] fp32 in the device layout.

    Groups laid side by side; within group h (tiles ntg, tokens th=128*ntg),
    flat column off_h + c*th + i*P + q holds x[tok0 + q*ntg + i, c*P + p]
    at partition row p.
    """
    plan = _plans(ts)
    blocks = []
    tok0 = 0
    for ntg in plan:
        th = ntg * P
        a = xs[tok0:tok0 + th].reshape(P, ntg, DC, P)  # [q, i, c, p]
        a = a.transpose(3, 2, 1, 0)                    # [p, c, i, q]
        blocks.append(np.ascontiguousarray(a).reshape(P, DC * th))
        tok0 += th
    return np.concatenate(blocks, axis=1)


def prepare_in_maps(x, gate_weight, bias):
    x = np.asarray(x, dtype=np.float32)
    gw = np.asarray(gate_weight, dtype=np.float32)
    bb = np.ascontiguousarray(np.asarray(bias, dtype=np.float32)).reshape(1, E)

    ts = T // NCORES

    # W^T in device layout [P, DC, E]: [p, c, e] = W[e, c*P + p]
    wt = np.ascontiguousarray(gw.T.reshape(DC, P, E).transpose(1, 0, 2))
    wh = wt.astype(FP16_NP)
    wl = (wt - wh.astype(np.float32)).astype(FP16_NP)
    wpk = np.concatenate([wh, wl], axis=2).reshape(P, DC * 2 * E)
    w8 = (wt * W8_SCALE).astype(FP8_NP).reshape(P, DC * E)

    in_maps = []
    for cid in range(NCORES):
        xt = _tile_xt(x[cid * ts:(cid + 1) * ts], ts)
        xh = xt.astype(FP16_NP)
        xr = ((xt - xh.astype(np.float32)) * R_SCALE).astype(FP8_NP)
        in_maps.append({
            "xt_hi": xh,
            "xt_r8": xr,
            "wt_pk": wpk,
            "wt_8": w8,
            "bias": bb,
        })
    return in_maps


def kernel(x, gate_weight, bias):
    ts = T // NCORES
    nc = _get_nc(ts)
    in_maps = prepare_in_maps(x, gate_weight, bias)
    res = run_bass_kernel_spmd(nc, in_maps, core_ids=list(range(NCORES)))
    idx = np.concatenate([r["out_idx"] for r in res.results], axis=0)
    wts = np.concatenate([r["out_w"] for r in res.results], axis=0)
    return idx, wts
